# revision 8
# baseline (speedup 1.0000x reference)
# Trainium2 Bass kernel for MemEffAttentionRope (B=2, N=2048, C=1024, H=16, D=64).
#
# Sharding: tensor-parallel over heads — each of the 8 cores owns 2 heads for
# both batches. Per core: qkv projection (only its heads' weight rows), qk
# layernorm + rope, full attention for its 4 (batch, head) pairs. The attention
# output is resharded head-major -> token-major with TWO AllToAlls (one per
# batch, [8, 128, 256] each); the batch-0 collective is triggered mid-kernel so
# it overlaps batch-1 attention and absorbs inter-core launch skew (the
# baseline's single end-of-kernel AllToAll serialized ~130us of wait+transfer).
# Each core then computes the output projection for its two 256-token blocks
# (one per batch); the host reassembles.
#
# Matmuls run as float32r (full-rate fp32 mode); PE transposes as float32
# (exact). Softmax skips the max-subtraction: q,k are layernormed so
# |q.k|*scale <= ~10 and exp stays well inside fp32 range. The softmax
# denominator comes from a ones-column appended to the PV stationary operand;
# it is broadcast across partitions via a DRAM bounce on the sync queue — the
# gpsimd queue is left free for the collective triggers, which would otherwise
# stall the batch-1 normalizes behind the batch-0 collective.
import sys

sys.path.insert(0, "/opt/trn_rl_repo")

import numpy as np

B, N, C = 2, 2048, 1024
H, D = 16, 64
T = B * N
NCORES = 8
HPC = H // NCORES  # heads per core = 2
EPS = 1e-5
SCALE = D ** -0.5
TSLICE = T // NCORES  # tokens per core in the output = 512 (256 per batch)
TBLK = TSLICE // B    # 256-token block per batch

_BUILT = {}


def _build(skip_affine):
    key = ("nc", skip_affine)
    if key in _BUILT:
        return _BUILT[key]

    import concourse.bass as bass
    import concourse.mybir as mybir
    import concourse.tile as tile
    from concourse import bacc
    from concourse.masks import make_identity

    f32 = mybir.dt.float32
    f32r = mybir.dt.float32r
    AF = mybir.ActivationFunctionType
    ALU = mybir.AluOpType

    nc = bacc.Bacc(None, target_bir_lowering=False, debug=False)

    xT = nc.dram_tensor("xT", [C, T], f32r, kind="ExternalInput")
    wqk = nc.dram_tensor("wqk", [C, 3 * HPC * D], f32r, kind="ExternalInput")
    pwT = nc.dram_tensor("pwT", [C, C], f32r, kind="ExternalInput")
    cos4 = nc.dram_tensor("cos4", [N, 2 * D], f32, kind="ExternalInput")
    sin4 = nc.dram_tensor("sin4", [N, 2 * D], f32, kind="ExternalInput")
    wln = nc.dram_tensor("wln", [4 * D], f32, kind="ExternalInput")
    bln = nc.dram_tensor("bln", [4 * D], f32, kind="ExternalInput")
    pb = nc.dram_tensor("pb", [C], f32, kind="ExternalInput")
    out = nc.dram_tensor("out", [TSLICE, C], f32, kind="ExternalOutput")

    NTB = N // 128          # 16 token tiles per batch
    NCT = C // 128          # 8 contraction tiles
    QKW = 3 * HPC * D       # 384

    with tile.TileContext(nc) as tc:
        import contextlib

        stack = contextlib.ExitStack()
        with stack:
            consts = stack.enter_context(tc.tile_pool(name="consts", bufs=1))
            dram = stack.enter_context(tc.tile_pool(name="dram", bufs=2, space="DRAM"))
            inner = contextlib.ExitStack()
            persist = inner.enter_context(tc.tile_pool(name="persist", bufs=1))

            # ---- constants ----
            wqk_sb = consts.tile([128, NCT, QKW], f32r)
            for ct in range(NCT):
                nc.sync.dma_start(out=wqk_sb[:, ct], in_=wqk[ct * 128:(ct + 1) * 128, :])
            # cos/sin on the scalar queue: keeps the sync queue free so the
            # first x-tile DMAs (behind wqk) land ASAP and the PE starts early
            cs_sb = consts.tile([128, NTB, 128], f32)
            sn_sb = consts.tile([128, NTB, 128], f32)
            nc.scalar.dma_start(out=cs_sb, in_=cos4.rearrange("(t p) c -> p t c", p=128))
            nc.scalar.dma_start(out=sn_sb, in_=sin4.rearrange("(t p) c -> p t c", p=128))
            if not skip_affine:
                wln_sb = consts.tile([128, 256], f32)
                bln_sb = consts.tile([128, 256], f32)
                nc.gpsimd.dma_start(out=wln_sb, in_=bass.AP(tensor=wln, offset=0, ap=[[0, 128], [1, 256]]))
                nc.gpsimd.dma_start(out=bln_sb, in_=bass.AP(tensor=bln, offset=0, ap=[[0, 128], [1, 256]]))
            pb_sb = consts.tile([128, C], f32)
            nc.gpsimd.dma_start(out=pb_sb, in_=bass.AP(tensor=pb, offset=0, ap=[[0, 128], [1, C]]))
            ident = consts.tile([128, 128], f32)
            make_identity(nc, ident)
            eps_sb = consts.tile([128, 1], f32)
            nc.vector.memset(eps_sb, EPS)

            # ---- persistent per-batch tensors ----
            qT = [persist.tile([128, N], f32r, tag=f"qT{b}", name=f"qT{b}") for b in range(B)]
            kTz = [[persist.tile([128, N], f32r, tag=f"kTz{b}{h}", name=f"kTz{b}{h}")
                    for h in range(HPC)] for b in range(B)]
            zeros64 = consts.tile([64, N], f32)
            nc.vector.memset(zeros64, 0.0)
            for b in range(B):
                for h in range(HPC):
                    nc.vector.tensor_copy(out=kTz[b][h][(64 - 64 * h):(128 - 64 * h), :], in_=zeros64)
            vpv = [persist.tile([128, NTB, HPC, 128], f32r, tag=f"vpv{b}", name=f"vpv{b}") for b in range(B)]
            vinit = consts.tile([128, HPC, 128], f32)
            nc.vector.memset(vinit, 0.0)
            nc.vector.memset(vinit[:, 0, 64:65], 1.0)
            nc.vector.memset(vinit[:, 1, 32:33], 1.0)
            for b in range(B):
                for tt in range(NTB):
                    nc.vector.tensor_copy(out=vpv[b][:, tt], in_=vinit)
            o_sb = persist.tile([128, T], f32r)  # attn out, channel-major
            a2a_in = [dram.tile([NCORES, 128, TBLK], f32r, tag=f"a2ain{b}", bufs=1,
                                name=f"a2ain{b}")
                      for b in range(B)]
            a2a_out = [dram.tile([NCORES, 128, TBLK], f32r, tag=f"a2aout{b}", bufs=1,
                                 name=f"a2aout{b}")
                       for b in range(B)]
            # per-batch staging for stage-1 (two-sweep structure)
            stg_sh = persist.tile([128, NTB, 256], f32, name="stg_sh")
            stg_all = [stg_sh, stg_sh]
            mv_sh = persist.tile([128, NTB, 4, 2], f32, name="mv_sh")
            mv_all = [mv_sh, mv_sh]
            rstd_sh = persist.tile([128, NTB, 4], f32, name="rstd_sh")
            rstd_all = [rstd_sh, rstd_sh]

            s1 = inner.enter_context(tc.tile_pool(name="s1", bufs=4))
            xtp = inner.enter_context(tc.tile_pool(name="xt", bufs=2))
            ps_s1 = inner.enter_context(tc.tile_pool(name="ps_s1", bufs=2, space="PSUM"))
            ps_tp = ps_s1
            ps_st = inner.enter_context(tc.tile_pool(name="ps_st", bufs=2, space="PSUM"))
            ps_ot = inner.enter_context(tc.tile_pool(name="ps_ot", bufs=2, space="PSUM"))

            def s1_sweepA(b, gg):
                """qkv matmul + stats for 4 token tiles."""
                for half in range(2):
                    s1_sweepA_half(b, gg, half)

            def s1_sweepA_half(b, gg, half):
                col0 = b * N + gg * 512 + half * 256
                xt = xtp.tile([128, NCT, 256], f32r, tag="xt")
                for ct in range(NCT):
                    nc.sync.dma_start(
                        out=xt[:, ct],
                        in_=xT[ct * 128:(ct + 1) * 128, col0:col0 + 256])
                for sub in range(2):
                    tt = gg * 4 + half * 2 + sub
                    qkv_ps = ps_s1.tile([128, QKW], f32, tag="s1b", name="qkv_ps")
                    for ct in range(NCT):
                        nc.tensor.matmul(
                            qkv_ps,
                            xt[:, ct, sub * 128:(sub + 1) * 128],
                            wqk_sb[:, ct],
                            start=(ct == 0), stop=(ct == NCT - 1))
                    for h in range(HPC):
                        nc.vector.tensor_copy(
                            out=vpv[b][:, tt, h, 64 * h:64 * h + 64],
                            in_=qkv_ps[:, 256 + 64 * h:256 + 64 * (h + 1)])
                    stg = stg_all[b][:, tt]
                    nc.vector.tensor_copy(stg, qkv_ps[:, 0:256])
                    st6 = s1.tile([128, 4, 6], f32, tag="st6")
                    for g in range(4):
                        nc.vector.bn_stats(out=st6[:, g], in_=stg[:, g * 64:(g + 1) * 64])
                        nc.vector.bn_aggr(out=mv_all[b][:, tt, g], in_=st6[:, g])

            def s1_rstd(b):
                """one batched sqrt per batch: rstd = 1/sqrt(var + eps)"""
                nc.scalar.activation(
                    out=rstd_all[b][:],
                    in_=mv_all[b][:, :, :, 1],
                    func=AF.Sqrt, bias=eps_sb, scale=1.0)
                nc.vector.reciprocal_approx_fast(
                    out=rstd_all[b][:], in_=rstd_all[b][:])

            def s1_sweepB(b, gg):
                """normalize + rope + transpose for 4 token tiles (lag tp by 1)."""
                tps = []
                for sub in range(4):
                    tt = gg * 4 + sub
                    stg = stg_all[b][:, tt]
                    for g in range(4):
                        nc.vector.tensor_scalar(
                            out=stg[:, g * 64:(g + 1) * 64],
                            in0=stg[:, g * 64:(g + 1) * 64],
                            scalar1=mv_all[b][:, tt, g, 0:1],
                            scalar2=rstd_all[b][:, tt, g:g + 1],
                            op0=ALU.subtract, op1=ALU.mult)
                    if not skip_affine:
                        nc.vector.tensor_mul(stg, stg, wln_sb)
                        nc.vector.tensor_add(stg, stg, bln_sb)
                    xsw = s1.tile([128, 256], f32, tag="xsw", bufs=2)
                    xsw4 = xsw.rearrange("p (g two s) -> p g two s", g=4, two=2)
                    stg4 = stg.rearrange("p (g two s) -> p g two s", g=4, two=2)
                    nc.gpsimd.tensor_copy(out=xsw4[:, :, 0, :], in_=stg4[:, :, 1, :])
                    nc.gpsimd.tensor_copy(out=xsw4[:, :, 1, :], in_=stg4[:, :, 0, :])
                    nc.vector.tensor_mul(stg[:, 0:128], stg[:, 0:128], cs_sb[:, tt])
                    nc.vector.tensor_mul(stg[:, 128:256], stg[:, 128:256], cs_sb[:, tt])
                    nc.vector.tensor_mul(xsw[:, 0:128], xsw[:, 0:128], sn_sb[:, tt])
                    nc.vector.tensor_mul(xsw[:, 128:256], xsw[:, 128:256], sn_sb[:, tt])
                    nc.vector.tensor_add(stg, stg, xsw)
                    tps.append(tt)
                    if len(tps) > 1:
                        emit_tp(b, tps.pop(0))
                for tt in tps:
                    emit_tp(b, tt)

            def emit_tp(b, tt):
                stg = stg_all[b][:, tt]
                tpq = ps_tp.tile([128, 128], f32, tag="s1b", name="tpq")
                nc.tensor.transpose(tpq, stg[:, 0:128], ident)
                nc.vector.tensor_copy(out=qT[b][:, tt * 128:(tt + 1) * 128], in_=tpq)
                tpk = ps_tp.tile([128, 128], f32, tag="s1b", name="tpk")
                nc.tensor.transpose(tpk, stg[:, 128:256], ident)
                for h in range(HPC):
                    nc.vector.tensor_copy(
                        out=kTz[b][h][64 * h:64 * h + 64, tt * 128:(tt + 1) * 128],
                        in_=tpk[64 * h:64 * h + 64, :])

            def s2_unit(b, h, icp):
                """attention for one head, one pair of 512-col i-chunks.
                jt-outer so kT/vpv stationary tiles are reused across the pair;
                PV lags one jt behind ST so the PE never stalls on exp."""
                hp = h * 64
                ics = (2 * icp, 2 * icp + 1)
                ot_ps = {ic: ps_ot.tile([128, 512], f32, tag="ot", name=f"ot{b}{h}{ic}")
                         for ic in ics}
                pts = {}
                for jp in range(NTB // 2 + 1):
                    if jp < NTB // 2:
                        for ic in ics:
                            st_ps = ps_st.tile([128, 1024], f32, tag="st")
                            for half in range(2):
                                jt = 2 * jp + half
                                nc.tensor.matmul(
                                    st_ps[:, half * 512:(half + 1) * 512],
                                    kTz[b][h][:, jt * 128:(jt + 1) * 128],
                                    qT[b][:, ic * 512:(ic + 1) * 512],
                                    start=True, stop=True)
                            p_t = s1.tile([128, 1024], f32r, tag="pt")
                            nc.scalar.activation(out=p_t, in_=st_ps, func=AF.Exp,
                                                 scale=SCALE)
                            pts[(jp, ic)] = p_t
                    if jp > 0:
                        for ic in ics:
                            p_t = pts.pop((jp - 1, ic))
                            for half in range(2):
                                jt = 2 * (jp - 1) + half
                                nc.tensor.matmul(
                                    ot_ps[ic],
                                    vpv[b][:, jt, h, :],
                                    p_t[:, half * 512:(half + 1) * 512],
                                    start=(jp == 1 and half == 0),
                                    stop=(jp == NTB // 2 and half == 1))
                drow = 64 if h == 0 else 32
                for ic in ics:
                    rd = s1.tile([128, 512], f32, tag="rd", bufs=2)
                    nc.vector.tensor_copy(out=rd[drow:drow + 1, :], in_=ot_ps[ic][drow:drow + 1, :])
                    scr = dram.tile([512], f32, tag="scr")
                    nc.sync.dma_start(out=scr, in_=rd[drow:drow + 1, :])
                    bc = s1.tile([128, 512], f32, tag="bc", bufs=2)
                    nc.sync.dma_start(
                        out=bc,
                        in_=bass.AP(tensor=scr.tensor, offset=scr.offset,
                                    ap=[[0, 128]] + [list(x) for x in scr.ap]))
                    nc.vector.reciprocal_approx_fast(out=bc, in_=bc)
                    nc.vector.tensor_mul(
                        o_sb[hp:hp + 64, b * N + ic * 512:b * N + (ic + 1) * 512],
                        ot_ps[ic][hp:hp + 64, :], bc[hp:hp + 64, :])

            def stage_a2a(b, icp):
                """after both heads of (b, icp) are done, ship those token
                blocks to the per-batch all-to-all input buffer."""
                for m in range(4 * icp, 4 * icp + 4):
                    nc.sync.dma_start(
                        out=a2a_in[b][m],
                        in_=o_sb[:, b * N + m * TBLK:b * N + (m + 1) * TBLK])

            def trigger_a2a(b):
                nc.gpsimd.collective_compute(
                    "AllToAll",
                    mybir.AluOpType.bypass,
                    replica_groups=[list(range(NCORES))],
                    ins=[a2a_in[b].opt()],
                    outs=[a2a_out[b].opt()],
                )

            # ---------------- emission schedule ----------------
            for gg in range(4):
                s1_sweepA(0, gg)
            s1_rstd(0)
            for gg in range(4):
                s1_sweepB(0, gg)
            # overlap: batch-1 stage-1 (DVE-heavy) with batch-0 attention (PE-heavy)
            for gg in range(4):
                s1_sweepA(1, gg)
                s2_unit(0, gg % 2, gg // 2)
                if gg % 2 == 1:
                    stage_a2a(0, gg // 2)
            s1_rstd(1)
            for gg in range(4):
                s1_sweepB(1, gg)
            # batch-0 reshard starts now, overlapping batch-1 attention; it also
            # absorbs inter-core launch skew. (Emitted after sweepB so the
            # gpsimd queue has no compute work queued behind the trigger.)
            trigger_a2a(0)
            for u in range(4):
                s2_unit(1, u % 2, u // 2)
                if u % 2 == 1:
                    stage_a2a(1, u // 2)
            trigger_a2a(1)

            inner.close()
            # ---------------- stage 3: projection for both 256-blocks ----------------
            with tc.tile_pool(name="s3", bufs=1) as s3, \
                 tc.tile_pool(name="s3o", bufs=4) as s3o, \
                 tc.tile_pool(name="ps_pj", bufs=4, space="PSUM") as ps_pj:
                pwT_sb = s3.tile([128, NCT, C], f32r)
                for ct in range(NCT):
                    nc.sync.dma_start(out=pwT_sb[:, ct], in_=pwT[ct * 128:(ct + 1) * 128, :])
                otf = [[s3.tile([128, TBLK], f32r, tag=f"otf{b}{ct}", name=f"otf{b}{ct}")
                        for ct in range(NCT)] for b in range(B)]
                for b in range(B):
                    for ct in range(NCT):
                        nc.sync.dma_start(out=otf[b][ct][:], in_=a2a_out[b][ct])
                for b in range(B):
                    for ts_ in range(TBLK // 128):
                        row0 = b * TBLK + ts_ * 128
                        pp = {}
                        for oc in range(C // 512):
                            pp[oc] = ps_pj.tile([128, 512], f32, tag="pj", name=f"pj{b}{ts_}{oc}")
                        for ct in range(NCT):
                            for oc in range(C // 512):
                                nc.tensor.matmul(
                                    pp[oc],
                                    otf[b][ct][:, ts_ * 128:(ts_ + 1) * 128],
                                    pwT_sb[:, ct, oc * 512:(oc + 1) * 512],
                                    start=(ct == 0), stop=(ct == NCT - 1))
                        for oc in range(C // 512):
                            o_st = s3o.tile([128, 512], f32, tag="ost")
                            nc.vector.tensor_add(o_st, pp[oc], pb_sb[:, oc * 512:(oc + 1) * 512])
                            nc.sync.dma_start(
                                out=out[row0:row0 + 128, oc * 512:(oc + 1) * 512],
                                in_=o_st)

    nc.finalize()
    _BUILT[key] = nc
    return nc


def _host_prep(x, qkv_w, qn_w, qn_b, kn_w, kn_b, proj_w, proj_b, pos):
    x = np.asarray(x, dtype=np.float32)
    qkv_w = np.asarray(qkv_w, dtype=np.float32)
    proj_w = np.asarray(proj_w, dtype=np.float32)
    pos = np.asarray(pos)

    xT = np.ascontiguousarray(x.reshape(T, C).T)
    pwT = np.ascontiguousarray(proj_w.T)

    d2 = D // 2
    inv_freq = (np.float32(1.0) / (np.float32(10000.0) **
                (np.arange(d2, dtype=np.float32) / np.float32(d2)))).astype(np.float32)
    ang = pos.astype(np.float32)[:, None] * inv_freq[None, :]
    cos = np.cos(ang).astype(np.float32)
    sin = np.sin(ang).astype(np.float32)
    cos4 = np.ascontiguousarray(np.concatenate([cos, cos, cos, cos], axis=1))
    sin4 = np.ascontiguousarray(np.concatenate([-sin, sin, -sin, sin], axis=1))

    wln = np.ascontiguousarray(np.concatenate(
        [qn_w, qn_w, kn_w, kn_w]).astype(np.float32))
    bln = np.ascontiguousarray(np.concatenate(
        [qn_b, qn_b, kn_b, kn_b]).astype(np.float32))
    pb = np.ascontiguousarray(np.asarray(proj_b, dtype=np.float32))
    skip_affine = bool(np.all(wln == 1.0) and np.all(bln == 0.0))

    in_maps = []
    for k in range(NCORES):
        rows = slice(128 * k, 128 * (k + 1))
        wqk_k = np.ascontiguousarray(np.concatenate(
            [qkv_w[rows], qkv_w[C:][rows], qkv_w[2 * C:][rows]], axis=0).T)
        in_maps.append({
            "xT": xT, "wqk": wqk_k, "pwT": pwT,
            "cos4": cos4, "sin4": sin4,
            "wln": wln, "bln": bln, "pb": pb,
        })
    return in_maps, skip_affine


def run_on_device(inputs, trace=False):
    from concourse.bass_utils import run_bass_kernel_spmd

    in_maps, skip_affine = _host_prep(**inputs)
    nc = _build(skip_affine)
    res = run_bass_kernel_spmd(nc, in_maps, list(range(NCORES)), trace=trace)
    # core k's out rows [0:256) are batch-0 tokens [256k, 256k+256),
    # rows [256:512) are batch-1 tokens [256k, 256k+256).
    slices = [res.results[k]["out"] for k in range(NCORES)]
    b0 = np.concatenate([s[:TBLK] for s in slices], axis=0)
    b1 = np.concatenate([s[TBLK:] for s in slices], axis=0)
    return np.stack([b0, b1]).reshape(B, N, C), res


def kernel(**inputs):
    out, _ = run_on_device(inputs, trace=False)
    return out


# revision 10
# speedup vs baseline: 1.0332x; 1.0332x over previous
# Trainium2 Bass kernel for MemEffAttentionRope (B=2, N=2048, C=1024, H=16, D=64).
#
# Sharding: tensor-parallel over heads — each of the 8 cores owns 2 heads for
# both batches. Per core: qkv projection (only its heads' weight rows), qk
# layernorm + rope, full attention for its 4 (batch, head) pairs. The attention
# output is resharded head-major -> token-major with TWO AllToAlls (one per
# batch, [8, 128, 256] each); the batch-0 collective is triggered mid-kernel so
# it overlaps batch-1 attention and absorbs inter-core launch skew (the
# baseline's single end-of-kernel AllToAll serialized ~130us of wait+transfer).
# Each core then computes the output projection for its two 256-token blocks
# (one per batch); the host reassembles.
#
# Matmuls run as float32r (full-rate fp32 mode); PE transposes as float32
# (exact). Softmax skips the max-subtraction: q,k are layernormed so
# |q.k|*scale <= ~10 and exp stays well inside fp32 range. The softmax
# denominator comes from a ones-column appended to the PV stationary operand;
# it is broadcast across partitions via a DRAM bounce on the sync queue — the
# gpsimd queue is left free for the collective triggers, which would otherwise
# stall the batch-1 normalizes behind the batch-0 collective.
import sys

sys.path.insert(0, "/opt/trn_rl_repo")

import numpy as np

B, N, C = 2, 2048, 1024
H, D = 16, 64
T = B * N
NCORES = 8
HPC = H // NCORES  # heads per core = 2
EPS = 1e-5
SCALE = D ** -0.5
TSLICE = T // NCORES  # tokens per core in the output = 512 (256 per batch)
TBLK = TSLICE // B    # 256-token block per batch

_BUILT = {}


def _build(skip_affine):
    key = ("nc", skip_affine)
    if key in _BUILT:
        return _BUILT[key]

    import concourse.bass as bass
    import concourse.mybir as mybir
    import concourse.tile as tile
    from concourse import bacc
    from concourse.masks import make_identity

    f32 = mybir.dt.float32
    f32r = mybir.dt.float32r
    AF = mybir.ActivationFunctionType
    ALU = mybir.AluOpType

    nc = bacc.Bacc(None, target_bir_lowering=False, debug=False)

    xT = nc.dram_tensor("xT", [C, T], f32r, kind="ExternalInput")
    wqk = nc.dram_tensor("wqk", [C, 3 * HPC * D], f32r, kind="ExternalInput")
    pwT = nc.dram_tensor("pwT", [C, C], f32r, kind="ExternalInput")
    cos4 = nc.dram_tensor("cos4", [N, 2 * D], f32, kind="ExternalInput")
    sin4 = nc.dram_tensor("sin4", [N, 2 * D], f32, kind="ExternalInput")
    wln = nc.dram_tensor("wln", [4 * D], f32, kind="ExternalInput")
    bln = nc.dram_tensor("bln", [4 * D], f32, kind="ExternalInput")
    pb = nc.dram_tensor("pb", [C], f32, kind="ExternalInput")
    out = nc.dram_tensor("out", [TSLICE, C], f32, kind="ExternalOutput")

    NTB = N // 128          # 16 token tiles per batch
    NCT = C // 128          # 8 contraction tiles
    QKW = 3 * HPC * D       # 384

    with tile.TileContext(nc) as tc:
        import contextlib

        stack = contextlib.ExitStack()
        with stack:
            consts = stack.enter_context(tc.tile_pool(name="consts", bufs=1))
            dram = stack.enter_context(tc.tile_pool(name="dram", bufs=2, space="DRAM"))
            inner = contextlib.ExitStack()
            persist = inner.enter_context(tc.tile_pool(name="persist", bufs=1))

            # ---- constants ----
            wqk_sb = consts.tile([128, NCT, QKW], f32r)
            for ct in range(NCT):
                nc.sync.dma_start(out=wqk_sb[:, ct], in_=wqk[ct * 128:(ct + 1) * 128, :])
            # cos/sin on the scalar queue: keeps the sync queue free so the
            # first x-tile DMAs (behind wqk) land ASAP and the PE starts early
            cs_sb = consts.tile([128, NTB, 128], f32)
            sn_sb = consts.tile([128, NTB, 128], f32)
            nc.scalar.dma_start(out=cs_sb, in_=cos4.rearrange("(t p) c -> p t c", p=128))
            nc.scalar.dma_start(out=sn_sb, in_=sin4.rearrange("(t p) c -> p t c", p=128))
            if not skip_affine:
                wln_sb = consts.tile([128, 256], f32)
                bln_sb = consts.tile([128, 256], f32)
                nc.gpsimd.dma_start(out=wln_sb, in_=bass.AP(tensor=wln, offset=0, ap=[[0, 128], [1, 256]]))
                nc.gpsimd.dma_start(out=bln_sb, in_=bass.AP(tensor=bln, offset=0, ap=[[0, 128], [1, 256]]))
            pb_sb = consts.tile([128, C], f32)
            nc.gpsimd.dma_start(out=pb_sb, in_=bass.AP(tensor=pb, offset=0, ap=[[0, 128], [1, C]]))
            ident = consts.tile([128, 128], f32)
            make_identity(nc, ident)
            eps_sb = consts.tile([128, 1], f32)
            nc.vector.memset(eps_sb, EPS)

            # ---- persistent per-batch tensors ----
            qT = [persist.tile([128, N], f32r, tag=f"qT{b}", name=f"qT{b}") for b in range(B)]
            kTz = [[persist.tile([128, N], f32r, tag=f"kTz{b}{h}", name=f"kTz{b}{h}")
                    for h in range(HPC)] for b in range(B)]
            zeros64 = consts.tile([64, N], f32)
            nc.vector.memset(zeros64, 0.0)
            for b in range(B):
                for h in range(HPC):
                    nc.vector.tensor_copy(out=kTz[b][h][(64 - 64 * h):(128 - 64 * h), :], in_=zeros64)
            vpv = [persist.tile([128, NTB, HPC, 128], f32r, tag=f"vpv{b}", name=f"vpv{b}") for b in range(B)]
            vinit = consts.tile([128, HPC, 128], f32)
            nc.vector.memset(vinit, 0.0)
            nc.vector.memset(vinit[:, 0, 64:65], 1.0)
            nc.vector.memset(vinit[:, 1, 32:33], 1.0)
            for b in range(B):
                for tt in range(NTB):
                    nc.vector.tensor_copy(out=vpv[b][:, tt], in_=vinit)
            o_sb = persist.tile([128, T], f32r)  # attn out, channel-major
            a2a_in = [dram.tile([NCORES, 128, TBLK], f32r, tag=f"a2ain{b}", bufs=1,
                                name=f"a2ain{b}")
                      for b in range(B)]
            a2a_out = [dram.tile([NCORES, 128, TBLK], f32r, tag=f"a2aout{b}", bufs=1,
                                 name=f"a2aout{b}")
                       for b in range(B)]
            # per-batch staging for stage-1 (two-sweep structure)
            stg_sh = persist.tile([128, NTB, 256], f32, name="stg_sh")
            stg_all = [stg_sh, stg_sh]
            mv_sh = persist.tile([128, NTB, 4, 2], f32, name="mv_sh")
            mv_all = [mv_sh, mv_sh]
            rstd_sh = persist.tile([128, NTB, 4], f32, name="rstd_sh")
            rstd_all = [rstd_sh, rstd_sh]

            s1 = inner.enter_context(tc.tile_pool(name="s1", bufs=4))
            xtp = inner.enter_context(tc.tile_pool(name="xt", bufs=2))
            ps_s1 = inner.enter_context(tc.tile_pool(name="ps_s1", bufs=2, space="PSUM"))
            ps_tp = ps_s1
            ps_st = inner.enter_context(tc.tile_pool(name="ps_st", bufs=2, space="PSUM"))
            ps_ot = inner.enter_context(tc.tile_pool(name="ps_ot", bufs=2, space="PSUM"))

            def s1_sweepA(b, gg):
                """qkv matmul + stats for 4 token tiles."""
                for half in range(2):
                    s1_sweepA_half(b, gg, half)

            def s1_sweepA_half(b, gg, half):
                col0 = b * N + gg * 512 + half * 256
                xt = xtp.tile([128, NCT, 256], f32r, tag="xt")
                for ct in range(NCT):
                    nc.sync.dma_start(
                        out=xt[:, ct],
                        in_=xT[ct * 128:(ct + 1) * 128, col0:col0 + 256])
                for sub in range(2):
                    tt = gg * 4 + half * 2 + sub
                    qkv_ps = ps_s1.tile([128, QKW], f32, tag="s1b", name="qkv_ps")
                    for ct in range(NCT):
                        nc.tensor.matmul(
                            qkv_ps,
                            xt[:, ct, sub * 128:(sub + 1) * 128],
                            wqk_sb[:, ct],
                            start=(ct == 0), stop=(ct == NCT - 1))
                    for h in range(HPC):
                        nc.vector.tensor_copy(
                            out=vpv[b][:, tt, h, 64 * h:64 * h + 64],
                            in_=qkv_ps[:, 256 + 64 * h:256 + 64 * (h + 1)])
                    stg = stg_all[b][:, tt]
                    nc.vector.tensor_copy(stg, qkv_ps[:, 0:256])
                    st6 = s1.tile([128, 4, 6], f32, tag="st6")
                    for g in range(4):
                        nc.vector.bn_stats(out=st6[:, g], in_=stg[:, g * 64:(g + 1) * 64])
                        nc.vector.bn_aggr(out=mv_all[b][:, tt, g], in_=st6[:, g])

            def s1_rstd_gg(b, gg):
                """per-group sqrt: rstd = 1/sqrt(var + eps) for 4 token tiles,
                so sweepB(gg) can start without waiting for all of sweepA."""
                nc.scalar.activation(
                    out=rstd_all[b][:, gg * 4:(gg + 1) * 4],
                    in_=mv_all[b][:, gg * 4:(gg + 1) * 4, :, 1],
                    func=AF.Sqrt, bias=eps_sb, scale=1.0)
                nc.vector.reciprocal_approx_fast(
                    out=rstd_all[b][:, gg * 4:(gg + 1) * 4],
                    in_=rstd_all[b][:, gg * 4:(gg + 1) * 4])

            def s1_sweepB(b, gg):
                """normalize + rope + transpose for 4 token tiles (lag tp by 1)."""
                tps = []
                for sub in range(4):
                    tt = gg * 4 + sub
                    stg = stg_all[b][:, tt]
                    for g in range(4):
                        nc.vector.tensor_scalar(
                            out=stg[:, g * 64:(g + 1) * 64],
                            in0=stg[:, g * 64:(g + 1) * 64],
                            scalar1=mv_all[b][:, tt, g, 0:1],
                            scalar2=rstd_all[b][:, tt, g:g + 1],
                            op0=ALU.subtract, op1=ALU.mult)
                    if not skip_affine:
                        nc.vector.tensor_mul(stg, stg, wln_sb)
                        nc.vector.tensor_add(stg, stg, bln_sb)
                    xsw = s1.tile([128, 256], f32, tag="xsw", bufs=2)
                    xsw4 = xsw.rearrange("p (g two s) -> p g two s", g=4, two=2)
                    stg4 = stg.rearrange("p (g two s) -> p g two s", g=4, two=2)
                    nc.gpsimd.tensor_copy(out=xsw4[:, :, 0, :], in_=stg4[:, :, 1, :])
                    nc.gpsimd.tensor_copy(out=xsw4[:, :, 1, :], in_=stg4[:, :, 0, :])
                    nc.vector.tensor_mul(stg[:, 0:128], stg[:, 0:128], cs_sb[:, tt])
                    nc.vector.tensor_mul(stg[:, 128:256], stg[:, 128:256], cs_sb[:, tt])
                    nc.vector.tensor_mul(xsw[:, 0:128], xsw[:, 0:128], sn_sb[:, tt])
                    nc.vector.tensor_mul(xsw[:, 128:256], xsw[:, 128:256], sn_sb[:, tt])
                    nc.vector.tensor_add(stg, stg, xsw)
                    tps.append(tt)
                    if len(tps) > 1:
                        emit_tp(b, tps.pop(0))
                for tt in tps:
                    emit_tp(b, tt)

            def emit_tp(b, tt):
                stg = stg_all[b][:, tt]
                tpq = ps_tp.tile([128, 128], f32, tag="s1b", name="tpq")
                nc.tensor.transpose(tpq, stg[:, 0:128], ident)
                nc.vector.tensor_copy(out=qT[b][:, tt * 128:(tt + 1) * 128], in_=tpq)
                tpk = ps_tp.tile([128, 128], f32, tag="s1b", name="tpk")
                nc.tensor.transpose(tpk, stg[:, 128:256], ident)
                for h in range(HPC):
                    nc.vector.tensor_copy(
                        out=kTz[b][h][64 * h:64 * h + 64, tt * 128:(tt + 1) * 128],
                        in_=tpk[64 * h:64 * h + 64, :])

            def s2_unit(b, h, icp):
                """attention for one head, one pair of 512-col i-chunks.
                jt-outer so kT/vpv stationary tiles are reused across the pair;
                PV lags one jt behind ST so the PE never stalls on exp."""
                hp = h * 64
                ics = (2 * icp, 2 * icp + 1)
                ot_ps = {ic: ps_ot.tile([128, 512], f32, tag="ot", name=f"ot{b}{h}{ic}")
                         for ic in ics}
                pts = {}
                for jp in range(NTB // 2 + 1):
                    if jp < NTB // 2:
                        for ic in ics:
                            st_ps = ps_st.tile([128, 1024], f32, tag="st")
                            for half in range(2):
                                jt = 2 * jp + half
                                nc.tensor.matmul(
                                    st_ps[:, half * 512:(half + 1) * 512],
                                    kTz[b][h][:, jt * 128:(jt + 1) * 128],
                                    qT[b][:, ic * 512:(ic + 1) * 512],
                                    start=True, stop=True)
                            p_t = s1.tile([128, 1024], f32r, tag="pt")
                            nc.scalar.activation(out=p_t, in_=st_ps, func=AF.Exp,
                                                 scale=SCALE)
                            pts[(jp, ic)] = p_t
                    if jp > 0:
                        for ic in ics:
                            p_t = pts.pop((jp - 1, ic))
                            for half in range(2):
                                jt = 2 * (jp - 1) + half
                                nc.tensor.matmul(
                                    ot_ps[ic],
                                    vpv[b][:, jt, h, :],
                                    p_t[:, half * 512:(half + 1) * 512],
                                    start=(jp == 1 and half == 0),
                                    stop=(jp == NTB // 2 and half == 1))
                drow = 64 if h == 0 else 32
                for ic in ics:
                    rd = s1.tile([128, 512], f32, tag="rd", bufs=2)
                    nc.vector.tensor_copy(out=rd[drow:drow + 1, :], in_=ot_ps[ic][drow:drow + 1, :])
                    scr = dram.tile([512], f32, tag="scr")
                    nc.sync.dma_start(out=scr, in_=rd[drow:drow + 1, :])
                    bc = s1.tile([128, 512], f32, tag="bc", bufs=2)
                    nc.sync.dma_start(
                        out=bc,
                        in_=bass.AP(tensor=scr.tensor, offset=scr.offset,
                                    ap=[[0, 128]] + [list(x) for x in scr.ap]))
                    nc.vector.reciprocal_approx_fast(out=bc, in_=bc)
                    nc.vector.tensor_mul(
                        o_sb[hp:hp + 64, b * N + ic * 512:b * N + (ic + 1) * 512],
                        ot_ps[ic][hp:hp + 64, :], bc[hp:hp + 64, :])

            def stage_a2a(b, icp):
                """after both heads of (b, icp) are done, ship those token
                blocks to the per-batch all-to-all input buffer."""
                for m in range(4 * icp, 4 * icp + 4):
                    nc.sync.dma_start(
                        out=a2a_in[b][m],
                        in_=o_sb[:, b * N + m * TBLK:b * N + (m + 1) * TBLK])

            def trigger_a2a(b):
                nc.gpsimd.collective_compute(
                    "AllToAll",
                    mybir.AluOpType.bypass,
                    replica_groups=[list(range(NCORES))],
                    ins=[a2a_in[b].opt()],
                    outs=[a2a_out[b].opt()],
                )

            # ---------------- emission schedule ----------------
            # batch-0 lead-in, software-pipelined: sweepB(g-1) DVE work
            # overlaps sweepA(g) matmuls
            for gg in range(4):
                s1_sweepA(0, gg)
                s1_rstd_gg(0, gg)
                if gg > 0:
                    s1_sweepB(0, gg - 1)
            s1_sweepB(0, 3)
            # overlap: batch-1 stage-1 (DVE-heavy, incl. its sweepB) with
            # batch-0 attention (PE/ACT-heavy)
            for gg in range(4):
                s1_sweepA(1, gg)
                s1_rstd_gg(1, gg)
                if gg > 0:
                    s1_sweepB(1, gg - 1)
                s2_unit(0, gg % 2, gg // 2)
                if gg % 2 == 1:
                    stage_a2a(0, gg // 2)
            s1_sweepB(1, 3)
            # batch-0 reshard starts now, overlapping batch-1 attention; it also
            # absorbs inter-core launch skew. (Emitted after sweepB so the
            # gpsimd queue has no compute work queued behind the trigger.)
            trigger_a2a(0)
            for u in range(4):
                s2_unit(1, u % 2, u // 2)
                if u % 2 == 1:
                    stage_a2a(1, u // 2)
            trigger_a2a(1)

            inner.close()
            # ---------------- stage 3: projection for both 256-blocks ----------------
            with tc.tile_pool(name="s3", bufs=1) as s3, \
                 tc.tile_pool(name="s3o", bufs=4) as s3o, \
                 tc.tile_pool(name="ps_pj", bufs=4, space="PSUM") as ps_pj:
                pwT_sb = s3.tile([128, NCT, C], f32r)
                for ct in range(NCT):
                    nc.sync.dma_start(out=pwT_sb[:, ct], in_=pwT[ct * 128:(ct + 1) * 128, :])
                otf = [[s3.tile([128, TBLK], f32r, tag=f"otf{b}{ct}", name=f"otf{b}{ct}")
                        for ct in range(NCT)] for b in range(B)]
                for b in range(B):
                    for ct in range(NCT):
                        nc.sync.dma_start(out=otf[b][ct][:], in_=a2a_out[b][ct])
                for b in range(B):
                    for ts_ in range(TBLK // 128):
                        row0 = b * TBLK + ts_ * 128
                        pp = {}
                        for oc in range(C // 512):
                            pp[oc] = ps_pj.tile([128, 512], f32, tag="pj", name=f"pj{b}{ts_}{oc}")
                        for ct in range(NCT):
                            for oc in range(C // 512):
                                nc.tensor.matmul(
                                    pp[oc],
                                    otf[b][ct][:, ts_ * 128:(ts_ + 1) * 128],
                                    pwT_sb[:, ct, oc * 512:(oc + 1) * 512],
                                    start=(ct == 0), stop=(ct == NCT - 1))
                        for oc in range(C // 512):
                            o_st = s3o.tile([128, 512], f32, tag="ost")
                            nc.vector.tensor_add(o_st, pp[oc], pb_sb[:, oc * 512:(oc + 1) * 512])
                            nc.sync.dma_start(
                                out=out[row0:row0 + 128, oc * 512:(oc + 1) * 512],
                                in_=o_st)

    nc.finalize()
    _BUILT[key] = nc
    return nc


def _host_prep(x, qkv_w, qn_w, qn_b, kn_w, kn_b, proj_w, proj_b, pos):
    x = np.asarray(x, dtype=np.float32)
    qkv_w = np.asarray(qkv_w, dtype=np.float32)
    proj_w = np.asarray(proj_w, dtype=np.float32)
    pos = np.asarray(pos)

    xT = np.ascontiguousarray(x.reshape(T, C).T)
    pwT = np.ascontiguousarray(proj_w.T)

    d2 = D // 2
    inv_freq = (np.float32(1.0) / (np.float32(10000.0) **
                (np.arange(d2, dtype=np.float32) / np.float32(d2)))).astype(np.float32)
    ang = pos.astype(np.float32)[:, None] * inv_freq[None, :]
    cos = np.cos(ang).astype(np.float32)
    sin = np.sin(ang).astype(np.float32)
    cos4 = np.ascontiguousarray(np.concatenate([cos, cos, cos, cos], axis=1))
    sin4 = np.ascontiguousarray(np.concatenate([-sin, sin, -sin, sin], axis=1))

    wln = np.ascontiguousarray(np.concatenate(
        [qn_w, qn_w, kn_w, kn_w]).astype(np.float32))
    bln = np.ascontiguousarray(np.concatenate(
        [qn_b, qn_b, kn_b, kn_b]).astype(np.float32))
    pb = np.ascontiguousarray(np.asarray(proj_b, dtype=np.float32))
    skip_affine = bool(np.all(wln == 1.0) and np.all(bln == 0.0))

    in_maps = []
    for k in range(NCORES):
        rows = slice(128 * k, 128 * (k + 1))
        wqk_k = np.ascontiguousarray(np.concatenate(
            [qkv_w[rows], qkv_w[C:][rows], qkv_w[2 * C:][rows]], axis=0).T)
        in_maps.append({
            "xT": xT, "wqk": wqk_k, "pwT": pwT,
            "cos4": cos4, "sin4": sin4,
            "wln": wln, "bln": bln, "pb": pb,
        })
    return in_maps, skip_affine


def run_on_device(inputs, trace=False):
    from concourse.bass_utils import run_bass_kernel_spmd

    in_maps, skip_affine = _host_prep(**inputs)
    nc = _build(skip_affine)
    res = run_bass_kernel_spmd(nc, in_maps, list(range(NCORES)), trace=trace)
    # core k's out rows [0:256) are batch-0 tokens [256k, 256k+256),
    # rows [256:512) are batch-1 tokens [256k, 256k+256).
    slices = [res.results[k]["out"] for k in range(NCORES)]
    b0 = np.concatenate([s[:TBLK] for s in slices], axis=0)
    b1 = np.concatenate([s[TBLK:] for s in slices], axis=0)
    return np.stack([b0, b1]).reshape(B, N, C), res


def kernel(**inputs):
    out, _ = run_on_device(inputs, trace=False)
    return out


# revision 15
# speedup vs baseline: 1.0603x; 1.0262x over previous
# Trainium2 Bass kernel for MemEffAttentionRope (B=2, N=2048, C=1024, H=16, D=64).
#
# Sharding: tensor-parallel over heads — each of the 8 cores owns 2 heads for
# both batches. Per core: qkv projection (only its heads' weight rows), qk
# layernorm + rope, full attention for its 4 (batch, head) pairs. The attention
# output is resharded head-major -> token-major with TWO AllToAlls (one per
# batch, [8, 128, 256] each); the batch-0 collective is triggered mid-kernel so
# it overlaps batch-1 attention and absorbs inter-core launch skew (the
# baseline's single end-of-kernel AllToAll serialized ~130us of wait+transfer).
# Each core then computes the output projection for its two 256-token blocks
# (one per batch); the host reassembles.
#
# Matmuls run as float32r (full-rate fp32 mode); PE transposes as float32
# (exact). Softmax skips the max-subtraction: q,k are layernormed so
# |q.k|*scale <= ~10 and exp stays well inside fp32 range. The softmax
# denominator comes from a ones-column appended to the PV stationary operand;
# it is broadcast across partitions via a DRAM bounce on the sync queue — the
# gpsimd queue is left free for the collective triggers, which would otherwise
# stall the batch-1 normalizes behind the batch-0 collective.
import sys

sys.path.insert(0, "/opt/trn_rl_repo")

import numpy as np

B, N, C = 2, 2048, 1024
H, D = 16, 64
T = B * N
NCORES = 8
HPC = H // NCORES  # heads per core = 2
EPS = 1e-5
SCALE = D ** -0.5
TSLICE = T // NCORES  # tokens per core in the output = 512 (256 per batch)
TBLK = TSLICE // B    # 256-token block per batch

_BUILT = {}


def _build(skip_affine):
    key = ("nc", skip_affine)
    if key in _BUILT:
        return _BUILT[key]

    import concourse.bass as bass
    import concourse.mybir as mybir
    import concourse.tile as tile
    from concourse import bacc
    from concourse.masks import make_identity

    f32 = mybir.dt.float32
    f32r = mybir.dt.float32r
    AF = mybir.ActivationFunctionType
    ALU = mybir.AluOpType

    nc = bacc.Bacc(None, target_bir_lowering=False, debug=False)

    xT = nc.dram_tensor("xT", [C, T], f32r, kind="ExternalInput")
    wqk = nc.dram_tensor("wqk", [C, 3 * HPC * D], f32r, kind="ExternalInput")
    pwT = nc.dram_tensor("pwT", [C, C], f32r, kind="ExternalInput")
    cos4 = nc.dram_tensor("cos4", [N, 2 * D], f32, kind="ExternalInput")
    sin4 = nc.dram_tensor("sin4", [N, 2 * D], f32, kind="ExternalInput")
    wln = nc.dram_tensor("wln", [4 * D], f32, kind="ExternalInput")
    bln = nc.dram_tensor("bln", [4 * D], f32, kind="ExternalInput")
    pb = nc.dram_tensor("pb", [C], f32, kind="ExternalInput")
    out = nc.dram_tensor("out", [TSLICE, C], f32, kind="ExternalOutput")

    NTB = N // 128          # 16 token tiles per batch
    NCT = C // 128          # 8 contraction tiles
    QKW = 3 * HPC * D       # 384

    with tile.TileContext(nc) as tc:
        import contextlib

        stack = contextlib.ExitStack()
        with stack:
            consts = stack.enter_context(tc.tile_pool(name="consts", bufs=1))
            dram = stack.enter_context(tc.tile_pool(name="dram", bufs=2, space="DRAM"))
            inner = contextlib.ExitStack()
            persist = inner.enter_context(tc.tile_pool(name="persist", bufs=1))

            # ---- constants ----
            wqk_sb = consts.tile([128, NCT, QKW], f32r)
            for ct in range(NCT):
                nc.sync.dma_start(out=wqk_sb[:, ct], in_=wqk[ct * 128:(ct + 1) * 128, :])
            # cos/sin on the scalar queue: keeps the sync queue free so the
            # first x-tile DMAs (behind wqk) land ASAP and the PE starts early
            cs_sb = consts.tile([128, NTB, 128], f32)
            sn_sb = consts.tile([128, NTB, 128], f32)
            nc.scalar.dma_start(out=cs_sb, in_=cos4.rearrange("(t p) c -> p t c", p=128))
            nc.scalar.dma_start(out=sn_sb, in_=sin4.rearrange("(t p) c -> p t c", p=128))
            if not skip_affine:
                wln_sb = consts.tile([128, 256], f32)
                bln_sb = consts.tile([128, 256], f32)
                nc.gpsimd.dma_start(out=wln_sb, in_=bass.AP(tensor=wln, offset=0, ap=[[0, 128], [1, 256]]))
                nc.gpsimd.dma_start(out=bln_sb, in_=bass.AP(tensor=bln, offset=0, ap=[[0, 128], [1, 256]]))
            pb_sb = consts.tile([128, C], f32)
            nc.gpsimd.dma_start(out=pb_sb, in_=bass.AP(tensor=pb, offset=0, ap=[[0, 128], [1, C]]))
            ident = consts.tile([128, 128], f32)
            make_identity(nc, ident)
            eps_sb = consts.tile([128, 1], f32)
            nc.vector.memset(eps_sb, EPS)

            # ---- persistent per-batch tensors ----
            qT = [persist.tile([128, N], f32r, tag=f"qT{b}", name=f"qT{b}") for b in range(B)]
            kTz = [[persist.tile([128, N], f32r, tag=f"kTz{b}{h}", name=f"kTz{b}{h}")
                    for h in range(HPC)] for b in range(B)]
            zeros64 = consts.tile([64, N], f32)
            nc.vector.memset(zeros64, 0.0)
            for b in range(B):
                for h in range(HPC):
                    nc.vector.tensor_copy(out=kTz[b][h][(64 - 64 * h):(128 - 64 * h), :], in_=zeros64)
            vpv = [persist.tile([128, NTB, HPC, 128], f32r, tag=f"vpv{b}", name=f"vpv{b}") for b in range(B)]
            vinit = consts.tile([128, HPC, 128], f32)
            nc.vector.memset(vinit, 0.0)
            nc.vector.memset(vinit[:, 0, 64:65], 1.0)
            nc.vector.memset(vinit[:, 1, 32:33], 1.0)
            for b in range(B):
                for tt in range(NTB):
                    nc.vector.tensor_copy(out=vpv[b][:, tt], in_=vinit)
            o_sb = persist.tile([128, T], f32r)  # attn out, channel-major
            a2a_in = [[dram.tile([NCORES, 128, 128], f32r, tag=f"a2ain{b}{j}", bufs=1,
                                 name=f"a2ain{b}{j}")
                       for j in range(2)] for b in range(B)]
            a2a_out = [[dram.tile([NCORES, 128, 128], f32r, tag=f"a2aout{b}{j}", bufs=1,
                                  name=f"a2aout{b}{j}")
                        for j in range(2)] for b in range(B)]
            # per-batch staging for stage-1 (two-sweep structure)
            stg_sh = persist.tile([128, NTB, 256], f32, name="stg_sh")
            stg_all = [stg_sh, stg_sh]
            mv_sh = persist.tile([128, NTB, 4, 2], f32, name="mv_sh")
            mv_all = [mv_sh, mv_sh]
            rstd_sh = persist.tile([128, NTB, 4], f32, name="rstd_sh")
            rstd_all = [rstd_sh, rstd_sh]

            s1 = inner.enter_context(tc.tile_pool(name="s1", bufs=4))
            xtp = inner.enter_context(tc.tile_pool(name="xt", bufs=2))
            ps_s1 = inner.enter_context(tc.tile_pool(name="ps_s1", bufs=2, space="PSUM"))
            ps_tp = ps_s1
            ps_st = inner.enter_context(tc.tile_pool(name="ps_st", bufs=2, space="PSUM"))
            ps_ot = inner.enter_context(tc.tile_pool(name="ps_ot", bufs=2, space="PSUM"))

            def s1_sweepA(b, gg):
                """qkv matmul + stats for 4 token tiles."""
                for half in range(2):
                    s1_sweepA_half(b, gg, half)

            def s1_sweepA_half(b, gg, half):
                col0 = b * N + gg * 512 + half * 256
                xt = xtp.tile([128, NCT, 256], f32r, tag="xt")
                for ct in range(NCT):
                    nc.sync.dma_start(
                        out=xt[:, ct],
                        in_=xT[ct * 128:(ct + 1) * 128, col0:col0 + 256])
                for sub in range(2):
                    tt = gg * 4 + half * 2 + sub
                    qkv_ps = ps_s1.tile([128, QKW], f32, tag="s1b", name="qkv_ps")
                    for ct in range(NCT):
                        nc.tensor.matmul(
                            qkv_ps,
                            xt[:, ct, sub * 128:(sub + 1) * 128],
                            wqk_sb[:, ct],
                            start=(ct == 0), stop=(ct == NCT - 1))
                    for h in range(HPC):
                        nc.vector.tensor_copy(
                            out=vpv[b][:, tt, h, 64 * h:64 * h + 64],
                            in_=qkv_ps[:, 256 + 64 * h:256 + 64 * (h + 1)])
                    stg = stg_all[b][:, tt]
                    nc.vector.tensor_copy(stg, qkv_ps[:, 0:256])
                    st6 = s1.tile([128, 4, 6], f32, tag="st6")
                    for g in range(4):
                        nc.vector.bn_stats(out=st6[:, g], in_=stg[:, g * 64:(g + 1) * 64])
                        nc.vector.bn_aggr(out=mv_all[b][:, tt, g], in_=st6[:, g])

            def s1_rstd_gg(b, gg):
                """per-group sqrt: rstd = 1/sqrt(var + eps) for 4 token tiles,
                so sweepB(gg) can start without waiting for all of sweepA."""
                nc.scalar.activation(
                    out=rstd_all[b][:, gg * 4:(gg + 1) * 4],
                    in_=mv_all[b][:, gg * 4:(gg + 1) * 4, :, 1],
                    func=AF.Sqrt, bias=eps_sb, scale=1.0)
                nc.vector.reciprocal_approx_fast(
                    out=rstd_all[b][:, gg * 4:(gg + 1) * 4],
                    in_=rstd_all[b][:, gg * 4:(gg + 1) * 4])

            def s1_sweepB(b, gg):
                """normalize + rope + transpose for 4 token tiles (lag tp by 1)."""
                tps = []
                for sub in range(4):
                    tt = gg * 4 + sub
                    stg = stg_all[b][:, tt]
                    for g in range(4):
                        nc.vector.tensor_scalar(
                            out=stg[:, g * 64:(g + 1) * 64],
                            in0=stg[:, g * 64:(g + 1) * 64],
                            scalar1=mv_all[b][:, tt, g, 0:1],
                            scalar2=rstd_all[b][:, tt, g:g + 1],
                            op0=ALU.subtract, op1=ALU.mult)
                    if not skip_affine:
                        nc.vector.tensor_mul(stg, stg, wln_sb)
                        nc.vector.tensor_add(stg, stg, bln_sb)
                    xsw = s1.tile([128, 256], f32, tag="xsw", bufs=2)
                    xsw4 = xsw.rearrange("p (g two s) -> p g two s", g=4, two=2)
                    stg4 = stg.rearrange("p (g two s) -> p g two s", g=4, two=2)
                    nc.gpsimd.tensor_copy(out=xsw4[:, :, 0, :], in_=stg4[:, :, 1, :])
                    nc.gpsimd.tensor_copy(out=xsw4[:, :, 1, :], in_=stg4[:, :, 0, :])
                    nc.vector.tensor_mul(stg[:, 0:128], stg[:, 0:128], cs_sb[:, tt])
                    nc.vector.tensor_mul(stg[:, 128:256], stg[:, 128:256], cs_sb[:, tt])
                    nc.vector.tensor_mul(xsw[:, 0:128], xsw[:, 0:128], sn_sb[:, tt])
                    nc.vector.tensor_mul(xsw[:, 128:256], xsw[:, 128:256], sn_sb[:, tt])
                    nc.vector.tensor_add(stg, stg, xsw)
                    tps.append(tt)
                    if len(tps) > 1:
                        emit_tp(b, tps.pop(0))
                for tt in tps:
                    emit_tp(b, tt)

            def emit_tp(b, tt):
                stg = stg_all[b][:, tt]
                tpq = ps_tp.tile([128, 128], f32, tag="s1b", name="tpq")
                nc.tensor.transpose(tpq, stg[:, 0:128], ident)
                nc.vector.tensor_copy(out=qT[b][:, tt * 128:(tt + 1) * 128], in_=tpq)
                tpk = ps_tp.tile([128, 128], f32, tag="s1b", name="tpk")
                nc.tensor.transpose(tpk, stg[:, 128:256], ident)
                for h in range(HPC):
                    nc.vector.tensor_copy(
                        out=kTz[b][h][64 * h:64 * h + 64, tt * 128:(tt + 1) * 128],
                        in_=tpk[64 * h:64 * h + 64, :])

            def s2_unit(b, h, icp):
                """attention for one head, one pair of 512-col i-chunks.
                jt-outer so kT/vpv stationary tiles are reused across the pair;
                PV lags one jt behind ST so the PE never stalls on exp."""
                hp = h * 64
                ics = (2 * icp, 2 * icp + 1)
                ot_ps = {ic: ps_ot.tile([128, 512], f32, tag="ot", name=f"ot{b}{h}{ic}")
                         for ic in ics}
                pts = {}
                for jp in range(NTB // 2 + 1):
                    if jp < NTB // 2:
                        for ic in ics:
                            st_ps = ps_st.tile([128, 1024], f32, tag="st")
                            for half in range(2):
                                jt = 2 * jp + half
                                nc.tensor.matmul(
                                    st_ps[:, half * 512:(half + 1) * 512],
                                    kTz[b][h][:, jt * 128:(jt + 1) * 128],
                                    qT[b][:, ic * 512:(ic + 1) * 512],
                                    start=True, stop=True)
                            p_t = s1.tile([128, 1024], f32r, tag="pt")
                            nc.scalar.activation(out=p_t, in_=st_ps, func=AF.Exp,
                                                 scale=SCALE)
                            pts[(jp, ic)] = p_t
                    if jp > 0:
                        for ic in ics:
                            p_t = pts.pop((jp - 1, ic))
                            for half in range(2):
                                jt = 2 * (jp - 1) + half
                                nc.tensor.matmul(
                                    ot_ps[ic],
                                    vpv[b][:, jt, h, :],
                                    p_t[:, half * 512:(half + 1) * 512],
                                    start=(jp == 1 and half == 0),
                                    stop=(jp == NTB // 2 and half == 1))
                drow = 64 if h == 0 else 32
                for ic in ics:
                    rd = s1.tile([128, 512], f32, tag="rd", bufs=2)
                    nc.vector.tensor_copy(out=rd[drow:drow + 1, :], in_=ot_ps[ic][drow:drow + 1, :])
                    scr = dram.tile([512], f32, tag="scr")
                    nc.sync.dma_start(out=scr, in_=rd[drow:drow + 1, :])
                    bc = s1.tile([128, 512], f32, tag="bc", bufs=2)
                    nc.sync.dma_start(
                        out=bc,
                        in_=bass.AP(tensor=scr.tensor, offset=scr.offset,
                                    ap=[[0, 128]] + [list(x) for x in scr.ap]))
                    nc.vector.reciprocal_approx_fast(out=bc, in_=bc)
                    nc.vector.tensor_mul(
                        o_sb[hp:hp + 64, b * N + ic * 512:b * N + (ic + 1) * 512],
                        ot_ps[ic][hp:hp + 64, :], bc[hp:hp + 64, :])

            def stage_a2a(b, j):
                """after both heads of (b, icp=j) are done, ship that half-
                batch's eight 128-token blocks to its all-to-all input."""
                for m in range(NCORES):
                    c0 = b * N + j * 1024 + m * 128
                    nc.sync.dma_start(
                        out=a2a_in[b][j][m],
                        in_=o_sb[:, c0:c0 + 128])

            def trigger_a2a(b, j):
                nc.gpsimd.collective_compute(
                    "AllToAll",
                    mybir.AluOpType.bypass,
                    replica_groups=[list(range(NCORES))],
                    ins=[a2a_in[b][j].opt()],
                    outs=[a2a_out[b][j].opt()],
                )

            # ---------------- emission schedule ----------------
            # batch-0 lead-in, software-pipelined: sweepB(g-1) DVE work
            # overlaps sweepA(g) matmuls
            for gg in range(4):
                s1_sweepA(0, gg)
                s1_rstd_gg(0, gg)
                if gg > 0:
                    s1_sweepB(0, gg - 1)
            s1_sweepB(0, 3)
            # overlap: batch-1 stage-1 (DVE-heavy, incl. its sweepB) with
            # batch-0 attention (PE/ACT-heavy)
            for gg in range(4):
                s1_sweepA(1, gg)
                s1_rstd_gg(1, gg)
                if gg > 0:
                    s1_sweepB(1, gg - 1)
                s2_unit(0, gg % 2, gg // 2)
                if gg % 2 == 1:
                    stage_a2a(0, gg // 2)
            s1_sweepB(1, 3)
            # batch-0 reshard starts now, overlapping batch-1 attention; it
            # also absorbs inter-core launch skew. (Emitted after sweepB so
            # the gpsimd queue has no compute work queued behind the
            # triggers.) Batch 1's first half triggers mid-attention so only
            # its second half (0.5MB) remains in the tail.
            trigger_a2a(0, 0)
            trigger_a2a(0, 1)
            for u in range(4):
                s2_unit(1, u % 2, u // 2)
                if u % 2 == 1:
                    stage_a2a(1, u // 2)
                    trigger_a2a(1, u // 2)

            inner.close()
            # ---------------- stage 3: projection for both 256-blocks ----------------
            with tc.tile_pool(name="s3", bufs=1) as s3, \
                 tc.tile_pool(name="s3o", bufs=4) as s3o, \
                 tc.tile_pool(name="ps_pj", bufs=4, space="PSUM") as ps_pj:
                pwT_sb = s3.tile([128, NCT, C], f32r)
                for ct in range(NCT):
                    nc.sync.dma_start(out=pwT_sb[:, ct], in_=pwT[ct * 128:(ct + 1) * 128, :])
                otf = [[[s3.tile([128, 128], f32r, tag=f"otf{b}{j}{ct}", name=f"otf{b}{j}{ct}")
                         for ct in range(NCT)] for j in range(2)] for b in range(B)]
                for b in range(B):
                    for j in range(2):
                        for ct in range(NCT):
                            nc.sync.dma_start(out=otf[b][j][ct][:], in_=a2a_out[b][j][ct])
                for b in range(B):
                    for j in range(2):
                        row0 = (b * 2 + j) * 128
                        pp = {}
                        for oc in range(C // 512):
                            pp[oc] = ps_pj.tile([128, 512], f32, tag="pj", name=f"pj{b}{j}{oc}")
                        for ct in range(NCT):
                            for oc in range(C // 512):
                                nc.tensor.matmul(
                                    pp[oc],
                                    otf[b][j][ct][:],
                                    pwT_sb[:, ct, oc * 512:(oc + 1) * 512],
                                    start=(ct == 0), stop=(ct == NCT - 1))
                        for oc in range(C // 512):
                            o_st = s3o.tile([128, 512], f32, tag="ost")
                            nc.vector.tensor_add(o_st, pp[oc], pb_sb[:, oc * 512:(oc + 1) * 512])
                            nc.sync.dma_start(
                                out=out[row0:row0 + 128, oc * 512:(oc + 1) * 512],
                                in_=o_st)

    nc.finalize()
    _BUILT[key] = nc
    return nc


def _host_prep(x, qkv_w, qn_w, qn_b, kn_w, kn_b, proj_w, proj_b, pos):
    x = np.asarray(x, dtype=np.float32)
    qkv_w = np.asarray(qkv_w, dtype=np.float32)
    proj_w = np.asarray(proj_w, dtype=np.float32)
    pos = np.asarray(pos)

    xT = np.ascontiguousarray(x.reshape(T, C).T)
    pwT = np.ascontiguousarray(proj_w.T)

    d2 = D // 2
    inv_freq = (np.float32(1.0) / (np.float32(10000.0) **
                (np.arange(d2, dtype=np.float32) / np.float32(d2)))).astype(np.float32)
    ang = pos.astype(np.float32)[:, None] * inv_freq[None, :]
    cos = np.cos(ang).astype(np.float32)
    sin = np.sin(ang).astype(np.float32)
    cos4 = np.ascontiguousarray(np.concatenate([cos, cos, cos, cos], axis=1))
    sin4 = np.ascontiguousarray(np.concatenate([-sin, sin, -sin, sin], axis=1))

    wln = np.ascontiguousarray(np.concatenate(
        [qn_w, qn_w, kn_w, kn_w]).astype(np.float32))
    bln = np.ascontiguousarray(np.concatenate(
        [qn_b, qn_b, kn_b, kn_b]).astype(np.float32))
    pb = np.ascontiguousarray(np.asarray(proj_b, dtype=np.float32))
    skip_affine = bool(np.all(wln == 1.0) and np.all(bln == 0.0))

    in_maps = []
    for k in range(NCORES):
        rows = slice(128 * k, 128 * (k + 1))
        wqk_k = np.ascontiguousarray(np.concatenate(
            [qkv_w[rows], qkv_w[C:][rows], qkv_w[2 * C:][rows]], axis=0).T)
        in_maps.append({
            "xT": xT, "wqk": wqk_k, "pwT": pwT,
            "cos4": cos4, "sin4": sin4,
            "wln": wln, "bln": bln, "pb": pb,
        })
    return in_maps, skip_affine


def run_on_device(inputs, trace=False):
    from concourse.bass_utils import run_bass_kernel_spmd

    in_maps, skip_affine = _host_prep(**inputs)
    nc = _build(skip_affine)
    res = run_bass_kernel_spmd(nc, in_maps, list(range(NCORES)), trace=trace)
    # core k's out rows are four 128-token blocks:
    # [0:128)   = batch-0 tokens [128k, 128k+128)
    # [128:256) = batch-0 tokens [1024+128k, ...)
    # [256:384) = batch-1 tokens [128k, ...)
    # [384:512) = batch-1 tokens [1024+128k, ...)
    slices = [res.results[k]["out"] for k in range(NCORES)]
    halves = []
    for b in range(B):
        for j in range(2):
            r0 = (b * 2 + j) * 128
            halves.append(np.concatenate([s[r0:r0 + 128] for s in slices], axis=0))
    b0 = np.concatenate([halves[0], halves[1]], axis=0)
    b1 = np.concatenate([halves[2], halves[3]], axis=0)
    return np.stack([b0, b1]).reshape(B, N, C), res


def kernel(**inputs):
    out, _ = run_on_device(inputs, trace=False)
    return out
